# revision 72
# baseline (speedup 1.0000x reference)
"""Trainium2 Bass kernel for a dense transformer block (LN->causal attn->res->LN->MLP->res).

Shapes (hardcoded): x [2, 2048, 1024], 16 heads, head_dim 64, MLP hidden 4096, fp32 in/out.

Sharding: 8 cores = (batch b in {0,1}) x (query-tile stride j in {0..3}).
Core (b, j) owns the 512 queries at global 128-token tiles {j, 4+j, 8+j,
12+j} of batch b — the swizzle balances causal attention work exactly
across cores.  The per-core context is the batch's 2048 tokens SHIFTED
right by (3-j) tiles (zero-padded prefix), which puts the core's own
query tiles at fixed local positions {3, 7, 11, 15} so the SPMD program
is identical on every core.  Causality is enforced by suffix-N windows
plus per-core DATA only:
  * local key tile kt attends only query slots >= kt//4 (score matmuls
    over the column suffix [kt//4*128, 512) of Q^T),
  * a per-key additive bias (-30000 on the zero-padded prefix tiles)
    folded into the softmax exp on the ACT engine, and
  * a constant triangular mask multiplied onto the kt%4==3 (diagonal)
    key tiles' first 128-query block.
Each core computes LN1 + K/V over the whole context (data-parallel
replication), Q/attention/LN2/MLP only for its own 512 tokens, and
writes its [512, 1024] slice (slot-ordered) of the output.  No
cross-core communication (an AllGather variant that shards K/V
production 4-way was measured slower on this runtime).

Numerics: all matmul operands are bf16 with fp32 PSUM accumulation;
LN statistics and the residual stream stay fp32 (rel err ~1.5e-3; fp8
was evaluated and rejected at ~3e-2 vs the 2e-2 gate).

Attention is key-tile-outer, 8 rounds of head pairs: for each key tile,
S^T = K^T.T @ Q^T for the pair (row-packed 64+64 matmuls into a 2-bank
PSUM tile, each head's block bank-aligned), one paired exp on ACT, then
AV accumulation into per-head [65, 512] PSUM — the 65th V column is
all-ones and accumulates the softmax denominator.  K^T and V both stay
resident in SBUF (no DRAM bounce); K/V production is interleaved with
attention so the PE stays dense.
"""

from contextlib import ExitStack

import numpy as np

import concourse.bacc as bacc
import concourse.mybir as mybir
import concourse.tile as tile
from concourse.masks import make_identity

F32 = mybir.dt.float32
F32R = mybir.dt.float32r
BF16 = mybir.dt.bfloat16
AF = mybir.ActivationFunctionType
ALU = mybir.AluOpType

B = 2
T = 2048
D = 1024
H = 16
HD = 64
HDA = HD + 1  # +1 denominator column per head
MLP = 4096
NQ = 512  # tokens per core
CTX = T
EPS = 1e-5
NEG = -30000.0

N_CORES = 8
P = 128

PHASE_MARKS = []  # (label, id_watermark) for trace attribution


def _mark(nc, label):
    PHASE_MARKS.append((label, nc.next_id()))

KT_T = CTX // P  # 16 key tiles
D_T = D // P  # 8
Q_T = NQ // P  # 4
M_T = MLP // P  # 32
VA = H * HDA  # 1040 augmented V width


def build_program(loop_n: int = 1, bv_nonzero: bool = False):
    """Emit the SPMD Bass program. Returns finalized nc."""
    nc = bacc.Bacc("TRN2", target_bir_lowering=False)

    xc = nc.dram_tensor("xc", [CTX, D], F32, kind="ExternalInput")
    wqk = nc.dram_tensor("wqk", [D, 2 * D], BF16, kind="ExternalInput")
    bqk = nc.dram_tensor("bqk", [P, 2 * D_T], F32, kind="ExternalInput")
    wva = nc.dram_tensor("wva", [D, VA], BF16, kind="ExternalInput")
    bva = nc.dram_tensor("bva", [1, VA], BF16, kind="ExternalInput")
    biask = nc.dram_tensor("biask", [P, KT_T], F32, kind="ExternalInput")
    trimask = nc.dram_tensor("trimask", [P, 2 * P], BF16, kind="ExternalInput")
    wfc = nc.dram_tensor("wfc", [D, MLP], BF16, kind="ExternalInput")
    bfc = nc.dram_tensor("bfc", [P, M_T], F32, kind="ExternalInput")
    wproj = nc.dram_tensor("wproj", [MLP, D], BF16, kind="ExternalInput")
    bproj = nc.dram_tensor("bproj", [P, D_T], F32, kind="ExternalInput")
    out = nc.dram_tensor("out", [NQ, D], F32, kind="ExternalOutput")

    with tile.TileContext(nc) as tc:
        with ExitStack() as ctx:
            if loop_n > 1:
                ctx.enter_context(tc.For_i(0, loop_n, 1))
            const = ctx.enter_context(tc.tile_pool(name="const", bufs=1))
            identity = const.tile([P, P], F32)
            make_identity(nc, identity)
            identity_bf = const.tile([P, P], BF16)
            make_identity(nc, identity_bf)
            ones1 = const.tile([1, P], BF16)
            nc.vector.memset(ones1, 1.0)
            eps_t = const.tile([P, 1], F32)
            nc.vector.memset(eps_t, EPS)
            bqk_sb = const.tile([P, 2 * D_T], F32)
            nc.sync.dma_start(bqk_sb, bqk[:, :])
            bva_sb = const.tile([1, VA], BF16)
            nc.sync.dma_start(bva_sb, bva[:, :])
            biask_sb = const.tile([P, KT_T], F32)
            nc.sync.dma_start(biask_sb, biask[:, :])

            # Long-lived LEFT pools.  Everything hot (K^T, V, weight-stream
            # buffers) is allocated BEFORE P1's transient pools so no hot
            # write lands on a zone whose release waits on an earlier phase.
            xnt_cm = tc.tile_pool(name="xnt", bufs=1)
            xnt_pool = xnt_cm.__enter__()
            XN = xnt_pool.tile([P, D_T, CTX], BF16, name="XN")
            qt_cm = tc.tile_pool(name="qt", bufs=1)
            qt_pool = qt_cm.__enter__()
            QT = [qt_pool.tile([P, NQ], BF16, name=f"QT{i}") for i in range(D_T)]
            kt_cm = tc.tile_pool(name="ktsb", bufs=1)
            kt_pool = kt_cm.__enter__()
            KT = [kt_pool.tile([P, CTX], BF16, name=f"KT{i}") for i in range(D_T)]
            vsb_cm = tc.tile_pool(name="vsb", bufs=1)
            vsb_pool = vsb_cm.__enter__()
            VSB = [vsb_pool.tile([P, VA], BF16, name=f"VSB{i}") for i in range(KT_T)]
            wva_cm = tc.tile_pool(name="wvap", bufs=1)
            wva_pool = wva_cm.__enter__()
            wvasb = [wva_pool.tile([P, VA], BF16, name=f"wva{i}") for i in range(D_T)]
            p3tri_cm = tc.tile_pool(name="p3tri", bufs=1)
            p3tri = p3tri_cm.__enter__()
            tri_sb = p3tri.tile([P, 2, P], BF16)
            p2k_cm = tc.tile_pool(name="p2k", bufs=2)
            p2k = p2k_cm.__enter__()
            p2q_cm = tc.tile_pool(name="p2q", bufs=2)
            p2q = p2q_cm.__enter__()

            # RIGHT pools (live into P4/P5)
            yt_pool = ctx.enter_context(tc.tile_pool(name="yt", bufs=1, side="right"))
            YT = yt_pool.tile([P, D_T, NQ], BF16, name="YT")
            x2_pool = ctx.enter_context(tc.tile_pool(name="x2", bufs=1, side="right"))
            X2 = [x2_pool.tile([P, D], F32, name=f"X2{i}") for i in range(Q_T)]
            l2t_pool = ctx.enter_context(
                tc.tile_pool(name="l2t", bufs=1, side="right")
            )
            L2T = l2t_pool.tile([P, D_T, NQ], BF16, name="L2T")

            # PSUM for Q (bottom, released after P2a) and P1 transposes
            # (above it, released after P1): the attention score pool reuses
            # exactly this space later.
            p2qps_cm = tc.tile_pool(name="p2qps", bufs=3, space="PSUM")
            p2qps = p2qps_cm.__enter__()

            _mark(nc, "P1_ln1")
            # ---------------- P1: load x, LN1, transpose -> xnT (bf16) ---------
            # Query tiles (local 3,7,11,15) first so Q^T production can finish
            # early; then the rest in causal-usefulness order.
            p1_order = [3, 0, 1, 2, 7, 11, 15, 4, 5, 6, 8, 9, 10, 12, 13, 14]
            with tc.tile_pool(name="p1work", bufs=3) as p1w, tc.tile_pool(
                name="p1stat", bufs=4
            ) as p1s, tc.tile_pool(name="p1ps", bufs=2, space="PSUM") as p1ps:
                for tt in p1_order:
                    xt = p1w.tile([P, D], F32, tag="xt")
                    nc.sync.dma_start(xt, xc[tt * P : (tt + 1) * P, :])
                    stats = p1s.tile([P, 2, 6], F32, tag="stats")
                    for g in range(2):
                        nc.vector.bn_stats(
                            stats[:, g, :], xt[:, g * 512 : (g + 1) * 512]
                        )
                    mv = p1s.tile([P, 2], F32, tag="mv")
                    nc.vector.bn_aggr(mv, stats)
                    sd = p1s.tile([P, 1], F32, tag="sd")
                    nc.scalar.activation(sd, mv[:, 1:2], AF.Sqrt, bias=eps_t)
                    rstd = p1s.tile([P, 1], F32, tag="rstd")
                    nc.vector.reciprocal(rstd, sd)
                    nmb = p1s.tile([P, 1], F32, tag="nmb")
                    nc.vector.tensor_scalar(
                        nmb, mv[:, 0:1], rstd, -1.0, ALU.mult, ALU.mult
                    )
                    xn = p1w.tile([P, D], BF16, tag="xn")
                    nc.scalar.activation(xn, xt, AF.Identity, bias=nmb, scale=rstd)
                    for g in range(2):
                        tp = p1ps.tile([P, 4, P], BF16, tag="tp")
                        for q in range(4):
                            dt_ = 4 * g + q
                            nc.tensor.transpose(
                                tp[:, q, :], xn[:, dt_ * P : (dt_ + 1) * P],
                                identity_bf,
                            )
                        dst = XN[:, 4 * g : 4 * g + 4, tt * P : (tt + 1) * P]
                        if g == 0:
                            nc.vector.tensor_copy(dst, tp)
                        else:
                            nc.scalar.copy(dst, tp)

            # wva/tri loads emitted after P1 so the x loads go first in the
            # DMA queues (their pools are allocated above, in the permanent
            # zone)
            for kt_ in range(D_T):
                nc.sync.dma_start(wvasb[kt_], wva[kt_ * P : (kt_ + 1) * P, :])
            nc.sync.dma_start(
                tri_sb, trimask.rearrange("p (a q) -> p a q", q=P)
            )

            _mark(nc, "P2a_Q")
            # ---------------- P2a: Q^T ----------------
            for mtc in range(D_T // 2):
                ws = p2q.tile([P, D_T, 2 * P], BF16, tag="wsq")
                nc.sync.dma_start(
                    ws,
                    wqk[:, mtc * 2 * P : (mtc + 1) * 2 * P].rearrange(
                        "(a p) c -> p a c", p=P
                    ),
                )
                for half in range(2):
                    mt = 2 * mtc + half
                    ps = p2qps.tile([P, NQ], F32, tag="ps")
                    for kt_ in range(D_T):
                        nc.tensor.matmul(
                            ps,
                            ws[:, kt_, half * P : (half + 1) * P],
                            XN[:, kt_, :].rearrange("p (g q) -> p g q", q=P)[
                                :, 3::4, :
                            ],
                            start=(kt_ == 0),
                            stop=(kt_ == D_T - 1),
                        )
                    nc.scalar.activation(
                        QT[mt], ps, AF.Identity, bias=bqk_sb[:, mt : mt + 1]
                    )
            p2q_cm.__exit__(None, None, None)
            p2qps_cm.__exit__(None, None, None)

            _mark(nc, "P2c_V")
            # ------------- attention pools + K^T/V production ---------------
            # These pools land on the zones freed by P1/P2a work pools, whose
            # releases happen early; the hot K^T/V/weight buffers are in the
            # permanent section above and never wait.
            ptp_cm = tc.tile_pool(name="ptp", bufs=4)
            ptp = ptp_cm.__enter__()
            p3s_cm = tc.tile_pool(name="p3s", bufs=2)
            p3s = p3s_cm.__enter__()
            stps_cm = tc.tile_pool(name="stps", bufs=2, space="PSUM")
            stps = stps_cm.__enter__()
            yps_cm = tc.tile_pool(name="yps", bufs=2, space="PSUM")
            yps = yps_cm.__enter__()

            _mark(nc, "P2b_K01")
            # K chunks and V quarters share one 2-bank PSUM pool (budget:
            # stps 4 + yps 2 + this 2 = 8) and are interleaved so attention
            # round hp has KT[hp] and the V tiles it needs slightly ahead of
            # consumption.
            p2kps_cm = tc.tile_pool(name="p2kps", bufs=2, space="PSUM")
            p2kps = p2kps_cm.__enter__()

            def emit_k_chunk(mtc):
                ws = p2k.tile([P, D_T, 2 * P], BF16, tag="wsk")
                nc.sync.dma_start(
                    ws,
                    wqk[:, D + mtc * 2 * P : D + (mtc + 1) * 2 * P].rearrange(
                        "(a p) c -> p a c", p=P
                    ),
                )
                for half in range(2):
                    mt = 2 * mtc + half
                    for nt in range(CTX // 512):
                        ps = p2kps.tile([P, 512], F32, tag="ps")
                        for kt_ in range(D_T):
                            nc.tensor.matmul(
                                ps,
                                ws[:, kt_, half * P : (half + 1) * P],
                                XN[:, kt_, nt * 512 : (nt + 1) * 512],
                                start=(kt_ == 0),
                                stop=(kt_ == D_T - 1),
                            )
                        nc.vector.tensor_scalar_add(
                            KT[mt][:, nt * 512 : (nt + 1) * 512],
                            ps,
                            bqk_sb[:, D_T + mt : D_T + mt + 1],
                        )

            _mark(nc, "P2c_Vb")
            vchunks = [(0, 512), (512, 512), (1024, VA - 1024)]

            def emit_v_tile(mt):
                for c0, cw in vchunks:
                    ps = p2kps.tile([P, 512], F32, tag="ps")
                    for kt_ in range(D_T):
                        nc.tensor.matmul(
                            ps[:, :cw],
                            XN[:, kt_, mt * P : (mt + 1) * P],
                            wvasb[kt_][:, c0 : c0 + cw],
                            start=(kt_ == 0),
                            stop=(kt_ == D_T - 1 and not bv_nonzero),
                        )
                    if bv_nonzero:
                        nc.tensor.matmul(
                            ps[:, :cw],
                            ones1,
                            bva_sb[:, c0 : c0 + cw],
                            start=False,
                            stop=True,
                        )
                    nc.vector.tensor_copy(VSB[mt][:, c0 : c0 + cw], ps[:, :cw])
                if not bv_nonzero:
                    ones_cols = VSB[mt].rearrange("p (h c) -> p h c", c=HDA)[
                        :, :, HD : HD + 1
                    ]
                    nc.vector.memset(ones_cols, 1.0)

            _mark(nc, "P2b_K27")
            # ------------ interleaved K^T chunks + V quarters ---------------
            # K chunk hp//2 arrives just before rounds 2*mtc..2*mtc+1 need it
            # and V quarters fill in between; the PE stays dense through the
            # whole production+attention window (which is PE-throughput
            # bound at ~200us of matmul work).
            for mtc in range(D_T // 2):
                emit_k_chunk(mtc)
                for mt in range(4 * mtc, 4 * mtc + 4):
                    emit_v_tile(mt)
            p2kps_cm.__exit__(None, None, None)

            _mark(nc, "P3_attn")
            # P3: key-tile-outer attention, 8 rounds x 2 heads.  Suffix-N:
            # local key tile kt only scores against query slots >= kt//4
            # (columns [kt//4*128, 512) of Q^T).  The kt%4==3 tile is the
            # diagonal of slot kt//4: its first 128-query block gets the
            # triangular mask.
            for hp in range(H // 2):
                yp = [
                    yps.tile([HDA, NQ], F32, name=f"yp{hp}_{s}", tag="yp")
                    for s in range(2)
                ]
                for kt in range(KT_T):
                    qoff = (kt // 4) * P
                    nw = NQ - qoff
                    # head s at fixed bank-aligned offset s*NQ (a matmul
                    # output may not cross a PSUM bank boundary)
                    st2 = stps.tile([P, 2, NQ], F32, tag="st2")
                    for s in range(2):
                        nc.tensor.matmul(
                            st2[:, s, :nw],
                            KT[hp][s * HD : (s + 1) * HD, kt * P : (kt + 1) * P],
                            QT[hp][s * HD : (s + 1) * HD, qoff:],
                            start=True,
                            stop=True,
                            tile_position=(s * HD, 0),
                        )
                    pt2 = ptp.tile([P, 2, NQ], BF16, tag="pt2")
                    nc.scalar.activation(
                        pt2[:, :, :nw],
                        st2[:, :, :nw],
                        AF.Exp,
                        bias=biask_sb[:, kt : kt + 1],
                    )
                    if kt % 4 == 3:
                        pt2v = pt2[:, :, :P]
                        nc.vector.tensor_mul(pt2v, pt2v, tri_sb)
                    for s in range(2):
                        h = 2 * hp + s
                        nc.tensor.matmul(
                            yp[s][:, qoff:],
                            VSB[kt][:, h * HDA : (h + 1) * HDA],
                            pt2[:, s, :nw],
                            start=(kt == 0),
                            stop=(kt == KT_T - 1),
                        )
                for s in range(2):
                    ysb = p3s.tile([HDA, NQ], F32, name=f"ysb{hp}_{s}", tag="ysb")
                    nc.vector.tensor_copy(ysb, yp[s])
                    recip = p3s.tile([1, NQ], F32, tag="recip")
                    nc.vector.reciprocal(recip, ysb[HD : HD + 1, :])
                    rb = p3s.tile([HD, NQ], F32, tag="rb")
                    nc.gpsimd.partition_broadcast(rb, recip)
                    nc.vector.tensor_mul(
                        YT[s * HD : (s + 1) * HD, hp, :], ysb[:HD, :], rb
                    )

            p3s_cm.__exit__(None, None, None)
            ptp_cm.__exit__(None, None, None)
            p2k_cm.__exit__(None, None, None)
            p3tri_cm.__exit__(None, None, None)
            wva_cm.__exit__(None, None, None)
            yps_cm.__exit__(None, None, None)
            stps_cm.__exit__(None, None, None)
            vsb_cm.__exit__(None, None, None)
            kt_cm.__exit__(None, None, None)
            qt_cm.__exit__(None, None, None)
            xnt_cm.__exit__(None, None, None)

            _mark(nc, "P4_ln2")
            # ---------------- P4: residual + LN2 + transpose ----------------
            with tc.tile_pool(name="p4w", bufs=3) as p4w, tc.tile_pool(
                name="p4s", bufs=4
            ) as p4s, tc.tile_pool(name="p4ps", bufs=4, space="PSUM") as p4ps:
                for tt in range(Q_T):
                    xl = p4w.tile([P, D], F32, tag="xl")
                    nc.sync.dma_start(
                        xl, xc[(4 * tt + 3) * P : (4 * tt + 4) * P, :]
                    )
                    for g in range(2):
                        tp = p4ps.tile([P, 4, P], BF16, tag="tp")
                        for q in range(4):
                            mt = 4 * g + q
                            nc.tensor.transpose(
                                tp[:, q, :], YT[:, mt, tt * P : (tt + 1) * P],
                                identity_bf,
                            )
                        nc.vector.tensor_add(
                            X2[tt][:, 4 * g * P : (4 * g + 4) * P],
                            xl[:, 4 * g * P : (4 * g + 4) * P],
                            tp.rearrange("p a c -> p (a c)"),
                        )
                    stats = p4s.tile([P, 2, 6], F32, tag="stats2")
                    for g in range(2):
                        nc.vector.bn_stats(
                            stats[:, g, :], X2[tt][:, g * 512 : (g + 1) * 512]
                        )
                    mv = p4s.tile([P, 2], F32, tag="mv2")
                    nc.vector.bn_aggr(mv, stats)
                    sd = p4s.tile([P, 1], F32, tag="sd2")
                    nc.scalar.activation(sd, mv[:, 1:2], AF.Sqrt, bias=eps_t)
                    rstd = p4s.tile([P, 1], F32, tag="rstd2")
                    nc.vector.reciprocal(rstd, sd)
                    nmb = p4s.tile([P, 1], F32, tag="nmb2")
                    nc.vector.tensor_scalar(
                        nmb, mv[:, 0:1], rstd, -1.0, ALU.mult, ALU.mult
                    )
                    l2 = p4w.tile([P, D], BF16, tag="l2")
                    nc.scalar.activation(l2, X2[tt], AF.Identity, bias=nmb, scale=rstd)
                    for g in range(2):
                        tp = p4ps.tile([P, 4, P], BF16, tag="tp")
                        for q in range(4):
                            mt = 4 * g + q
                            nc.tensor.transpose(
                                tp[:, q, :], l2[:, mt * P : (mt + 1) * P],
                                identity_bf,
                            )
                        nc.vector.tensor_copy(
                            L2T[:, 4 * g : 4 * g + 4, tt * P : (tt + 1) * P], tp
                        )

            _mark(nc, "P5_mlp")
            # ---------------- P5: MLP + final residual ----------------
            with tc.tile_pool(name="h1t", bufs=1) as h1t_pool, tc.tile_pool(
                name="p5w", bufs=2
            ) as p5w, tc.tile_pool(name="p5o", bufs=1) as p5o, tc.tile_pool(
                name="p5ps", bufs=2, space="PSUM"
            ) as p5ps, tc.tile_pool(
                name="p5tps", bufs=4, space="PSUM"
            ) as p5tps:
                bfc_sb = p5o.tile([P, M_T], F32)
                nc.sync.dma_start(bfc_sb, bfc[:, :])
                bproj_sb = p5o.tile([P, D_T], F32)
                nc.sync.dma_start(bproj_sb, bproj[:, :])
                OUT = [p5o.tile([P, D], F32, name=f"OUT{i}") for i in range(Q_T)]
                H1T = h1t_pool.tile([P, M_T, NQ], BF16, name="H1T")
                for mtc in range(M_T // 2):
                    ws = p5w.tile([P, D_T, 2 * P], BF16, tag="wsf")
                    nc.sync.dma_start(
                        ws,
                        wfc[:, mtc * 2 * P : (mtc + 1) * 2 * P].rearrange(
                            "(a p) c -> p a c", p=P
                        ),
                    )
                    for half in range(2):
                        mt = 2 * mtc + half
                        ps = p5ps.tile([P, NQ], F32, tag="ps")
                        for kt_ in range(D_T):
                            nc.tensor.matmul(
                                ps,
                                ws[:, kt_, half * P : (half + 1) * P],
                                L2T[:, kt_, :],
                                start=(kt_ == 0),
                                stop=(kt_ == D_T - 1),
                            )
                        nc.vector.tensor_scalar(
                            H1T[:, mt, :], ps, bfc_sb[:, mt : mt + 1], 0.0,
                            ALU.add, ALU.max,
                        )
                for mtc in range(D_T // 2):
                    ws = p5w.tile([P, M_T, 2 * P], BF16, tag="wsp")
                    nc.sync.dma_start(
                        ws,
                        wproj[:, mtc * 2 * P : (mtc + 1) * 2 * P].rearrange(
                            "(a p) c -> p a c", p=P
                        ),
                    )
                    mlp2 = p5w.tile([P, 2, NQ], BF16, tag="mlpt")
                    for half in range(2):
                        mt = 2 * mtc + half
                        ps = p5ps.tile([P, NQ], F32, tag="ps")
                        for kt_ in range(M_T):
                            nc.tensor.matmul(
                                ps,
                                ws[:, kt_, half * P : (half + 1) * P],
                                H1T[:, kt_, :],
                                start=(kt_ == 0),
                                stop=(kt_ == M_T - 1),
                            )
                        nc.vector.tensor_scalar_add(
                            mlp2[:, half, :], ps, bproj_sb[:, mt : mt + 1]
                        )
                    for tt in range(Q_T):
                        tp = p5tps.tile([P, 2, P], BF16, tag="tp")
                        for half in range(2):
                            nc.tensor.transpose(
                                tp[:, half, :],
                                mlp2[:, half, tt * P : (tt + 1) * P],
                                identity_bf,
                            )
                        nc.vector.tensor_add(
                            OUT[tt][:, 2 * mtc * P : (2 * mtc + 2) * P],
                            X2[tt][:, 2 * mtc * P : (2 * mtc + 2) * P],
                            tp.rearrange("p a c -> p (a c)"),
                        )
                for tt in range(Q_T):
                    nc.sync.dma_start(out[tt * P : (tt + 1) * P, :], OUT[tt])

    nc.finalize()
    return nc


_PROG = {}


def _get_program(bv_nonzero: bool = False):
    if bv_nonzero not in _PROG:
        _PROG[bv_nonzero] = build_program(bv_nonzero=bv_nonzero)
    return _PROG[bv_nonzero]


def make_in_maps(x, ln1_scale, ln1_shift, w_qkv, b_qkv, ln2_scale, ln2_shift,
                 w_fc, b_fc, w_proj, b_proj):
    """Host-side prep: fold LN affine into weights, prescale Q by 1/sqrt(hd),
    augment V with an all-ones output column per head, build per-core rotated
    context + causal bias/mask data."""
    import ml_dtypes

    bf16 = ml_dtypes.bfloat16

    x = np.asarray(x, np.float32)
    ln1_scale = np.asarray(ln1_scale, np.float32)
    ln1_shift = np.asarray(ln1_shift, np.float32)
    w_qkv = np.asarray(w_qkv, np.float32)
    b_qkv = np.asarray(b_qkv, np.float32)
    ln2_scale = np.asarray(ln2_scale, np.float32)
    ln2_shift = np.asarray(ln2_shift, np.float32)
    w_fc = np.asarray(w_fc, np.float32)
    b_fc = np.asarray(b_fc, np.float32)
    w_proj = np.asarray(w_proj, np.float32)
    b_proj = np.asarray(b_proj, np.float32)

    # fold LN1 affine into qkv weights
    w1 = ln1_scale[:, None] * w_qkv  # [D, 3D]
    b1 = b_qkv + ln1_shift @ w_qkv  # [3D]
    sc = 1.0 / np.sqrt(HD)
    wq = w1[:, :D] * sc
    bq = b1[:D] * sc
    wk = w1[:, D : 2 * D]
    bk = b1[D : 2 * D]
    wv = w1[:, 2 * D :]
    bv = b1[2 * D :]

    wqk_h = np.ascontiguousarray(
        np.concatenate([wq, wk], axis=1).astype(bf16)
    )  # [D, 2D] bf16
    bqk_h = np.ascontiguousarray(
        np.concatenate([bq, bk]).reshape(2 * D_T, P).T
    )  # [128, 16] f32

    wva_h = np.zeros((D, VA), np.float32)
    bva_h = np.zeros((1, VA), np.float32)
    for h in range(H):
        wva_h[:, h * HDA : h * HDA + HD] = wv[:, h * HD : (h + 1) * HD]
        bva_h[0, h * HDA : h * HDA + HD] = bv[h * HD : (h + 1) * HD]
        bva_h[0, h * HDA + HD] = 1.0  # denominator ones column
    wva_h = wva_h.astype(bf16)
    bva_h = bva_h.astype(bf16)

    # fold LN2 affine into fc
    wfc_h = np.ascontiguousarray(ln2_scale[:, None] * w_fc).astype(bf16)
    wproj_h = np.ascontiguousarray(w_proj).astype(bf16)
    bfc_h = np.ascontiguousarray((b_fc + ln2_shift @ w_fc).reshape(M_T, P).T)
    bproj_h = np.ascontiguousarray(b_proj.reshape(D_T, P).T)  # [128, 8]

    # triangular mask for the diagonal key tile (S^T orientation: tri[k, q] =
    # k <= q within a 128x128 tile), duplicated for the head-pair layout
    kk = np.arange(P)[:, None]
    qq = np.arange(P)[None, :]
    tri = (kk <= qq).astype(np.float32)  # [128, 128]
    tri_h = np.ascontiguousarray(np.concatenate([tri, tri], axis=1)).astype(
        bf16
    )  # [128, 2*128]

    in_maps = []
    for c in range(N_CORES):
        b, j = divmod(c, 4)
        xb = x[b]  # [T, D]
        # Local context = global tokens shifted right by (3-j) tiles of 128:
        # local q-tile 4t+3 holds global q-tile 4t+j; local key tile k holds
        # global key tile k-(3-j).  The (3-j)-tile prefix is zero padding,
        # masked out via the per-key additive bias.
        sh = (3 - j) * P
        xperm = np.concatenate([np.zeros((sh, D), np.float32), xb[: T - sh]])
        bias = np.zeros(CTX, np.float32)
        bias[:sh] = NEG
        biask_h = np.ascontiguousarray(bias.reshape(KT_T, P).T)  # [128, 16]
        in_maps.append(
            {
                "xc": np.ascontiguousarray(xperm),
                "wqk": wqk_h,
                "bqk": bqk_h,
                "wva": wva_h,
                "bva": bva_h,
                "biask": biask_h,
                "trimask": tri_h,
                "wfc": wfc_h,
                "bfc": bfc_h,
                "wproj": wproj_h,
                "bproj": bproj_h,
            }
        )
    return in_maps


def assemble_output(results):
    out = np.empty((B, T, D), np.float32)
    for c in range(N_CORES):
        b, j = divmod(c, 4)
        r = results[c]["out"]  # [512, 1024]: slot t rows -> global tile 4t+j
        for t in range(Q_T):
            g = 4 * t + j
            out[b, g * P : (g + 1) * P, :] = r[t * P : (t + 1) * P]
    return out


def kernel(**inputs) -> np.ndarray:
    from concourse.bass_utils import run_bass_kernel_spmd

    in_maps = make_in_maps(**inputs)
    bva = np.asarray(in_maps[0]["bva"], np.float32)[0]
    mask = np.ones(VA, bool)
    mask[HD::HDA] = False  # the ones columns
    nc = _get_program(bv_nonzero=bool(np.any(bva[mask] != 0.0)))
    res = run_bass_kernel_spmd(nc, in_maps, core_ids=list(range(N_CORES)))
    return assemble_output(res.results)

